# revision 13
# baseline (speedup 1.0000x reference)
"""Mesa-layer memory kernel for Trainium2 (8 NeuronCores, data-parallel over B).

Math: the reference's T-step Sherman-Morrison / discounted-accumulation
recurrence has a closed form,
    R_final = (I + K^T K)^{-1}            (eps term is O(1e-6) relative)
    S_final^T = K^T diag(c) V,   c_t = prod_{s>t} gamma_s
so per memory b the output is out_b = Q_b @ (R_b @ S_b^T).
R is computed with Newton-Schulz iterations in residual form
    X <- X + X^T (I - A X)
(bf16 iterations + one fp32 refinement; A = I + K^T K has cond ~3, so one
refinement lands at ~1e-5, far below the bf16 readout floor of ~3e-3).

v2 architecture — DMA-saturation pipeline. The kernel is HBM-bound:
33.6 MB/core at the measured ~425 GB/s per-core fabric rate = ~79 us
floor, so the design keeps the DMA queues streaming end to end:
  * sync queue carries ALL input loads (25.2 MB), interleaved
    K0 V0 K1 V1 K2 V2 Q0 K3 V3 Q1 ... K7 V7 Q5 Q6 Q7 — each memory's
    K/V lead its Q by two memories, so the NS inversion of the last pair
    hides under the final Q loads. The sync engine issues nothing else,
    so no pool-rotation wait can ever deadlock the queue feed.
  * scalar queue carries the 8 per-memory output stores; they stream
    from ~25 us onward, sharing the HBM pipe with the load tail.
  * Casts: K and V(scaled) -> bf16 on Scalar (into the combined kvb
    tile), Q -> bf16 on GpSimd (the only engine with slack; its 5.4 us
    per cast hides under the 10 us per-memory arrival spacing), except
    the last two Q casts which split Scalar/DVE halves to shorten the
    tail. The discount scaling is an in-place fp32 DVE multiply.
  * Emission is woven: each pair-group's NS iterations interleave with
    the next memories' A/S accumulation matmuls and earlier memories'
    readout chunks, so the PE (~70 us of matmul) never sits on a serial
    chain and the three non-PE engines each stay near ~45-50 us.

Layout: timestep t maps to (partition p, slot r) via t = 16 p + r; every
HBM transfer is 8 KB/partition contiguous. The suffix cumprod of gammas
runs in log space: 16-step free-dim scans + one triangular matmul for the
cross-partition prefix.

Each core owns B/8 = 8 independent memories; no cross-core communication.
"""

import numpy as np

B, T, DK, DV, NQ = 64, 2048, 128, 128, 2048
NCORES = 8
BPC = B // NCORES          # memories per core
P = 128                    # partitions
R16 = T // P               # 16 row-slots per partition
GCLAMP = 1e-30             # gamma clamp before log (exact-0 gammas)

NS_BF = 5                  # Newton-Schulz iterations in bf16
NS_FP = 1                  # fp32 refinement iterations
NGRP = 4                   # NS pair-groups
GSZ = BPC // NGRP          # 2 memories per group


def build_nc(ns_bf=NS_BF, ns_fp=NS_FP):
    import itertools

    import concourse.mybir as mybir
    import concourse.tile as tile
    from concourse import bacc
    from concourse.masks import make_identity, make_upper_triangular

    fp32 = mybir.dt.float32
    bf16 = mybir.dt.bfloat16
    AF = mybir.ActivationFunctionType
    OP = mybir.AluOpType
    AX = mybir.AxisListType
    NIT = ns_bf + ns_fp

    nc = bacc.Bacc(trn_type="TRN2", target_bir_lowering=False, debug=False)
    keys = nc.dram_tensor("keys", [BPC, T, DK], fp32, kind="ExternalInput").ap()
    values = nc.dram_tensor("values", [BPC, T, DV], fp32, kind="ExternalInput").ap()
    gammas = nc.dram_tensor("gammas", [BPC, T], fp32, kind="ExternalInput").ap()
    queries = nc.dram_tensor("queries", [BPC, NQ, DK], fp32, kind="ExternalInput").ap()
    out = nc.dram_tensor("out", [BPC, NQ, DV], fp32, kind="ExternalOutput").ap()

    with tile.TileContext(nc) as tc:
        const = tc.alloc_tile_pool(name="const", bufs=1)
        gam = tc.alloc_tile_pool(name="gam", bufs=1)
        kp = tc.alloc_tile_pool(name="kp", bufs=3)
        vp = tc.alloc_tile_pool(name="vp", bufs=3)
        kvbp = tc.alloc_tile_pool(name="kvbp", bufs=3)
        qp = tc.alloc_tile_pool(name="qp", bufs=3)
        qbp = tc.alloc_tile_pool(name="qbp", bufs=4)
        qtp = tc.alloc_tile_pool(name="qtp", bufs=3)
        outp = tc.alloc_tile_pool(name="outp", bufs=3)
        small = tc.alloc_tile_pool(name="small", bufs=1)
        xs = tc.alloc_tile_pool(name="xs", bufs=2)
        ps_as = tc.alloc_tile_pool(name="ps_as", bufs=2, space="PSUM")
        ps_w = tc.alloc_tile_pool(name="ps_w", bufs=4, space="PSUM")
        ps_ro = tc.alloc_tile_pool(name="ps_ro", bufs=2, space="PSUM")

        ident = const.tile([P, P], fp32)
        make_identity(nc, ident)
        ident_bf = const.tile([P, P], bf16)
        make_identity(nc, ident_bf)
        # identity pair for the group-batched I - A@X residual
        ident2 = const.tile([P, GSZ * P], fp32)
        for i in range(GSZ):
            make_identity(nc, ident2[:, i * P : (i + 1) * P])
        # strict upper triangular and all-ones for the cross-partition
        # prefix-sum of per-partition gamma-log totals
        utri = const.tile([P, P], fp32)
        make_upper_triangular(nc, utri, val=1.0, diag=False)
        ones2 = const.tile([P, P], fp32)
        nc.gpsimd.memset(ones2[:], 1.0)

        # ---- phase 0: suffix cumprod of gammas (log space) ----
        # g16[p, i, r] = gamma[i, 16p + r]
        g16 = gam.tile([P, BPC, R16], fp32)
        nc.sync.dma_start(g16[:], gammas.rearrange("i (p r) -> p i r", r=R16))
        g16f = g16.rearrange("p i r -> p (i r)")
        nc.vector.tensor_scalar_max(g16f, g16f, GCLAMP)
        nc.scalar.activation(g16f, g16f, AF.Ln)
        incl = gam.tile([P, BPC, R16], fp32)
        zz = gam.tile([P, R16], fp32)
        nc.vector.memset(zz[:], 0.0)
        # joiner: make DVE observe the ACT (Ln) dependency before the scans
        joiner = gam.tile([P, 1], fp32)
        nc.vector.tensor_copy(out=joiner[:], in_=g16[:, 0, 0:1])
        for i in range(BPC):
            nc.vector.tensor_tensor_scan(
                incl[:, i, :], g16[:, i, :], zz[:], 0.0, OP.add, OP.add
            )
        # per-partition totals -> cross-partition exclusive prefix + full sum
        ptot = gam.tile([P, BPC], fp32)
        nc.vector.tensor_copy(out=ptot[:], in_=incl[:, :, R16 - 1])
        ps_pre = ps_w.tile([P, 2 * BPC], fp32, tag="w", name="ps_pre")
        nc.tensor.matmul(ps_pre[:, 0:BPC], utri[:], ptot[:])          # offs
        nc.tensor.matmul(ps_pre[:, BPC : 2 * BPC], ones2[:], ptot[:])  # total
        pre_sb = gam.tile([P, 2 * BPC], fp32)
        nc.vector.tensor_copy(out=pre_sb[:], in_=ps_pre[:])
        bias2 = gam.tile([P, BPC], fp32)
        nc.vector.tensor_tensor(
            bias2[:], pre_sb[:, BPC : 2 * BPC], pre_sb[:, 0:BPC], OP.subtract
        )
        # c_t[p, i, r] = exp(bias - incl) = prod_{s > 16p+r} gamma[i, s]
        c_t = gam.tile([P, BPC, R16], fp32)
        for i in range(BPC):
            nc.scalar.activation(
                c_t[:, i, :], incl[:, i, :], AF.Exp,
                bias=bias2[:, i : i + 1], scale=-1.0,
            )

        # ---- load emission: ALL inputs on the sync queue ----
        k_sb = [None] * BPC
        v_sb = [None] * BPC
        q_sb = [None] * BPC
        kvb = [None] * BPC
        Qb = [None] * BPC

        def load_k(i):
            k_sb[i] = kp.tile([P, R16, DK], fp32, tag="k", name=f"k{i}")
            nc.sync.dma_start(
                k_sb[i][:], keys[i].rearrange("(p r) k -> p r k", p=P)
            )

        def load_v(i):
            v_sb[i] = vp.tile([P, R16, DV], fp32, tag="v", name=f"v{i}")
            nc.sync.dma_start(
                v_sb[i][:], values[i].rearrange("(p r) k -> p r k", p=P)
            )

        def load_q(i):
            q_sb[i] = qp.tile([P, R16, DK], fp32, tag="q", name=f"q{i}")
            nc.sync.dma_start(
                q_sb[i][:], queries[i].rearrange("(p r) k -> p r k", p=P)
            )

        # sync queue: K0 V0 K1 V1 K2 V2 Q0 K3 V3 Q1 ... K7 V7 Q5 Q6 Q7
        load_k(0); load_v(0)
        load_k(1); load_v(1)
        load_k(2); load_v(2)
        load_q(0)
        for i in range(3, BPC):
            load_k(i); load_v(i)
            load_q(i - 3)
        load_q(5); load_q(6); load_q(7)

        # ---- per-memory state tiles ----
        A_sb = [small.tile([P, P], fp32, tag=f"A{i}", name=f"A{i}") for i in range(BPC)]
        A_bf = [small.tile([P, P], bf16, tag=f"Ab{i}", name=f"Ab{i}") for i in range(BPC)]
        ST_sb = [small.tile([P, P], fp32, tag=f"S{i}", name=f"S{i}") for i in range(BPC)]
        Phi_bf = [small.tile([P, P], bf16, tag=f"Pb{i}", name=f"Phib{i}") for i in range(BPC)]
        rs_sb = [small.tile([P, 1], fp32, tag=f"r{i}", name=f"rs{i}") for i in range(BPC)]
        Xg = [None] * NGRP

        def cast_q(i):
            """Q fp32 -> bf16. GpSimd for the bulk; last two split to
            Scalar/DVE halves to keep the pipeline tail short."""
            Qb[i] = qbp.tile([P, R16, DK], bf16, tag="qb", name=f"qb{i}")
            if i < BPC - 2:
                nc.gpsimd.tensor_copy(out=Qb[i][:], in_=q_sb[i][:])
            else:
                h = R16 // 2
                nc.scalar.copy(out=Qb[i][:, 0:h, :], in_=q_sb[i][:, 0:h, :])
                nc.vector.tensor_copy(
                    out=Qb[i][:, h:R16, :], in_=q_sb[i][:, h:R16, :]
                )
            yield

        def prep(i):
            """K cast, in-place cV mult, V cast, A/S accumulation.
            The mult and V cast run in halves so the cast (and then the
            S-half matmuls) start before the whole multiply finishes."""
            kvb[i] = kvbp.tile([P, R16, 2 * P], bf16, tag="kvb", name=f"kvb{i}")
            nc.scalar.copy(out=kvb[i][:, :, 0:DK], in_=k_sb[i][:])
            h = R16 // 2
            for lo, hi in ((0, h), (h, R16)):
                nc.vector.tensor_tensor(
                    v_sb[i][:, lo:hi, :], v_sb[i][:, lo:hi, :],
                    c_t[:, i, lo:hi, None].to_broadcast((P, h, DV)),
                    OP.mult,
                )
                yield
                nc.scalar.copy(
                    out=kvb[i][:, lo:hi, DK : 2 * P], in_=v_sb[i][:, lo:hi, :]
                )
            yield
            ps = ps_as.tile([P, 2 * P], fp32, tag="as", name=f"ps_as{i}")
            for r in range(R16):
                nc.tensor.matmul(
                    ps[:], kvb[i][:, r, 0:DK], kvb[i][:, r, :],
                    start=(r == 0), stop=(r == R16 - 1),
                )
                if r % 2 == 1:
                    yield
            nc.vector.tensor_tensor(A_sb[i][:], ps[:, 0:P], ident[:], OP.add)
            nc.vector.tensor_copy(out=ST_sb[i][:], in_=ps[:, P : 2 * P])
            nc.scalar.copy(out=A_bf[i][:], in_=A_sb[i][:])
            nc.vector.tensor_reduce(
                rs_sb[i][:], A_sb[i][:], AX.X, OP.add, apply_absolute_value=True
            )
            nc.vector.reciprocal(rs_sb[i][:], rs_sb[i][:])
            yield

        def x0(g):
            xw = xs.tile([P, GSZ * P], bf16, tag=f"Xb{g}", name=f"Xb{g}_0")
            for i in range(GSZ):
                nc.scalar.activation(
                    xw[:, i * P : (i + 1) * P], ident[:], AF.Copy,
                    scale=rs_sb[GSZ * g + i][:],
                )
            Xg[g] = xw

        def ns_group(g):
            """All NS iterations for pair-group g, yielding between stages."""
            for it in range(NIT):
                bf_iter = it < ns_bf
                last_bf = it == ns_bf - 1
                Amat = A_bf if bf_iter else A_sb
                pa = ps_w.tile([P, GSZ * P], fp32, tag="w", name=f"pa{g}_{it}")
                for i in range(GSZ):
                    sl = slice(i * P, (i + 1) * P)
                    nc.tensor.matmul(pa[:, sl], Amat[GSZ * g + i][:], Xg[g][:, sl])
                yield
                eg = xs.tile(
                    [P, GSZ * P], bf16 if bf_iter else fp32,
                    tag=f"e{g}_{bf_iter}", name=f"e{g}_{it}",
                )
                nc.vector.scalar_tensor_tensor(
                    eg[:], pa[:], -1.0, ident2[:], OP.mult, OP.add
                )
                yield
                pb = ps_w.tile([P, GSZ * P], fp32, tag="w", name=f"pb{g}_{it}")
                for i in range(GSZ):
                    sl = slice(i * P, (i + 1) * P)
                    nc.tensor.matmul(pb[:, sl], Xg[g][:, sl], eg[:, sl])
                yield
                out_fp32 = (not bf_iter) or last_bf
                xn = xs.tile(
                    [P, GSZ * P], fp32 if out_fp32 else bf16,
                    tag=f"Xf{g}" if out_fp32 else f"Xb{g}",
                    name=f"X{g}_{it + 1}",
                )
                nc.vector.tensor_tensor(xn[:], Xg[g][:], pb[:], OP.add)
                Xg[g] = xn
                yield

        def phi(i):
            g, sl = i // GSZ, slice((i % GSZ) * P, (i % GSZ + 1) * P)
            ps_phi = ps_w.tile([P, P], fp32, tag="w", name=f"ps_phi{i}")
            nc.tensor.matmul(ps_phi[:], Xg[g][:, sl], ST_sb[i][:])
            nc.scalar.copy(out=Phi_bf[i][:], in_=ps_phi[:])

        out_r = [out[i].rearrange("(p r) v -> p r v", p=P) for i in range(BPC)]

        def ro(i):
            """Readout: PE-transpose Q chunk, matmul vs Phi, copy, store."""
            qt = qtp.tile([P, R16, DK], bf16, tag="qt", name=f"qt{i}")
            o_sb = outp.tile([P, R16, DV], fp32, tag="o", name=f"o{i}")
            for c in range(4):
                ps_qt = ps_ro.tile([P, 4 * P], bf16, tag="ro", name=f"ps_qt{i}_{c}")
                for j in range(4):
                    nc.tensor.transpose(
                        ps_qt[:, j * P : (j + 1) * P], Qb[i][:, 4 * c + j, :],
                        ident_bf[:],
                    )
                yield
                nc.vector.tensor_copy(
                    out=qt[:, 4 * c : 4 * c + 4, :], in_=ps_qt[:]
                )
                yield
                ps_o = ps_ro.tile([P, 4 * P], fp32, tag="ro", name=f"ps_o{i}_{c}")
                for j in range(4):
                    nc.tensor.matmul(
                        ps_o[:, j * P : (j + 1) * P], qt[:, 4 * c + j, :],
                        Phi_bf[i][:],
                    )
                yield
                sl = slice(4 * c, 4 * c + 4)
                if c % 2 == 0:
                    nc.vector.tensor_copy(out=o_sb[:, sl, :], in_=ps_o[:])
                else:
                    nc.scalar.copy(out=o_sb[:, sl, :], in_=ps_o[:])
                yield
            nc.scalar.dma_start(out_r[i][:], o_sb[:])
            yield

        def weave(*gens):
            active = [iter(g) for g in gens]
            while active:
                for g in list(active):
                    try:
                        next(g)
                    except StopIteration:
                        active.remove(g)

        chain = itertools.chain

        # ---- woven emission ----
        for _ in prep(0):
            pass
        for _ in prep(1):
            pass
        x0(0)
        weave(ns_group(0), chain(prep(2), cast_q(0), prep(3), cast_q(1)))
        phi(0); phi(1); x0(1)
        weave(ns_group(1), chain(prep(4), cast_q(2), prep(5), cast_q(3)),
              ro(0), ro(1))
        phi(2); phi(3); x0(2)
        weave(ns_group(2), chain(prep(6), cast_q(4), prep(7), cast_q(5)),
              ro(2), ro(3))
        phi(4); phi(5); x0(3)
        weave(ns_group(3), chain(cast_q(6), cast_q(7)), ro(4), ro(5))
        phi(6); phi(7)
        weave(ro(6), ro(7))

        for pool in (ps_ro, ps_w, ps_as, xs, small, outp, qtp, qbp, qp, kvbp,
                     vp, kp, gam, const):
            pool.release()

    if not nc.is_finalized():
        nc.finalize()
    return nc


def kernel(**inputs) -> np.ndarray:
    keys = np.ascontiguousarray(inputs["keys"], dtype=np.float32)
    values = np.ascontiguousarray(inputs["values"], dtype=np.float32)
    gammas = np.ascontiguousarray(inputs["gammas"], dtype=np.float32)
    queries = np.ascontiguousarray(inputs["queries"], dtype=np.float32)

    from concourse.bass_utils import run_bass_kernel_spmd

    nc = build_nc()
    in_maps = []
    for m in range(NCORES):
        s = slice(m * BPC, (m + 1) * BPC)
        in_maps.append(
            {
                "keys": keys[s],
                "values": values[s],
                "gammas": gammas[s],
                "queries": queries[s],
            }
        )
    res = run_bass_kernel_spmd(nc, in_maps, core_ids=list(range(NCORES)))
    return np.concatenate([res.results[m]["out"] for m in range(NCORES)], axis=0)


# revision 18
# speedup vs baseline: 1.1484x; 1.1484x over previous
"""Mesa-layer memory kernel for Trainium2 (8 NeuronCores, data-parallel over B).

Math: the reference's T-step Sherman-Morrison / discounted-accumulation
recurrence has a closed form,
    R_final = (I + K^T K)^{-1}            (eps term is O(1e-6) relative)
    S_final^T = K^T diag(c) V,   c_t = prod_{s>t} gamma_s
so per memory b the output is out_b = Q_b @ (R_b @ S_b^T).
R is computed with Newton-Schulz iterations in residual form
    X <- X + X^T (I - A X)
(bf16 iterations + one fp32 refinement; A = I + K^T K has cond ~3, so one
refinement lands at ~1e-5, far below the bf16 readout floor of ~3e-3).

v4 architecture — DMA-saturation pipeline. The kernel is HBM-bound:
33.6 MB/core at the measured ~425 GB/s per-core fabric rate = ~79 us
floor, so the design keeps the DMA queues streaming end to end and gives
every compute engine slack against the ~9.5 us/memory arrival cadence:
  * Queries are sharded host-side in TRANSPOSED layout [DK, NQ] (pure
    layout choice, same bytes moved), so the readout's Q^T operand loads
    directly: no PE transposes, no transpose-PSUM copies, and the
    readout chain is just cast -> 16 matmuls -> 4 copies -> store.
  * sync queue carries ALL input loads, interleaved K0 V0 K1 V1 K2 V2
    Q0 K3 V3 Q1 ... so each memory's K/V lead its Q by two memories;
    the scalar queue carries the 8 output stores, streaming from ~20 us.
  * V is cast to bf16 FIRST, then scaled in place by bf16(c) on DVE
    (bf16 multiply runs the fast 16-bit path; the fp32-width multiply
    measured 2.3-8 us under SBUF contention, bf16 ~0.6 us).
  * A and S^T accumulate in two separate single-bank PSUM tiles (each
    [P,512] fp32 = exactly one 2 KB zero region, so the two interleaved
    accumulation groups can never zero each other).
  * Casts alternate Scalar/DVE by memory parity so each engine's
    in-order stream follows the data-arrival order.
  * Emission is woven: each pair-group's NS iterations interleave with
    the next memories' accumulation matmuls and earlier memories'
    readout chunks.

Layout: timestep t maps to (partition p, slot r) via t = 16 p + r; every
HBM transfer is 8 KB/partition contiguous. The suffix cumprod of gammas
runs in log space: 16-step free-dim scans + one triangular matmul for the
cross-partition prefix.

Each core owns B/8 = 8 independent memories; no cross-core communication.
"""

import numpy as np

B, T, DK, DV, NQ = 64, 2048, 128, 128, 2048
NCORES = 8
BPC = B // NCORES          # memories per core
P = 128                    # partitions
R16 = T // P               # 16 row-slots per partition
GCLAMP = 1e-30             # gamma clamp before log (exact-0 gammas)

NS_BF = 5                  # Newton-Schulz iterations in bf16
NS_FP = 1                  # fp32 refinement iterations
NGRP = 4                   # NS pair-groups
GSZ = BPC // NGRP          # 2 memories per group


def build_nc(ns_bf=NS_BF, ns_fp=NS_FP):
    import itertools

    import concourse.mybir as mybir
    import concourse.tile as tile
    from concourse import bacc
    from concourse.masks import make_identity, make_upper_triangular

    fp32 = mybir.dt.float32
    bf16 = mybir.dt.bfloat16
    AF = mybir.ActivationFunctionType
    OP = mybir.AluOpType
    AX = mybir.AxisListType
    NIT = ns_bf + ns_fp

    nc = bacc.Bacc(trn_type="TRN2", target_bir_lowering=False, debug=False)
    keys = nc.dram_tensor("keys", [BPC, T, DK], fp32, kind="ExternalInput").ap()
    values = nc.dram_tensor("values", [BPC, T, DV], fp32, kind="ExternalInput").ap()
    gammas = nc.dram_tensor("gammas", [BPC, T], fp32, kind="ExternalInput").ap()
    # host-transposed query layout: [DK, NQ] per memory
    queriesT = nc.dram_tensor("queriesT", [BPC, DK, NQ], fp32, kind="ExternalInput").ap()
    # blocked output layout: out_dev[i, m, s, v] = out[i, s*128 + m, v]
    # (the host un-blocks it; pure layout transform)
    out = nc.dram_tensor("out", [BPC, P, R16, DV], fp32, kind="ExternalOutput").ap()

    with tile.TileContext(nc) as tc:
        const = tc.alloc_tile_pool(name="const", bufs=1)
        gam = tc.alloc_tile_pool(name="gam", bufs=1)
        kp = tc.alloc_tile_pool(name="kp", bufs=3)
        vp = tc.alloc_tile_pool(name="vp", bufs=3)
        kbp = tc.alloc_tile_pool(name="kbp", bufs=3)
        vcbp = tc.alloc_tile_pool(name="vcbp", bufs=3)
        qp = tc.alloc_tile_pool(name="qp", bufs=3)
        qbp = tc.alloc_tile_pool(name="qbp", bufs=3)
        outp = tc.alloc_tile_pool(name="outp", bufs=3)
        small = tc.alloc_tile_pool(name="small", bufs=1)
        xs = tc.alloc_tile_pool(name="xs", bufs=2)
        ps_a = tc.alloc_tile_pool(name="ps_a", bufs=1, space="PSUM")
        ps_s = tc.alloc_tile_pool(name="ps_s", bufs=1, space="PSUM")
        ps_w = tc.alloc_tile_pool(name="ps_w", bufs=4, space="PSUM")
        ps_ro = tc.alloc_tile_pool(name="ps_ro", bufs=2, space="PSUM")

        ident = const.tile([P, P], fp32)
        make_identity(nc, ident)
        # identity pair for the group-batched I - A@X residual
        ident2 = const.tile([P, GSZ * P], fp32)
        for i in range(GSZ):
            make_identity(nc, ident2[:, i * P : (i + 1) * P])
        # strict upper triangular and all-ones for the cross-partition
        # prefix-sum of per-partition gamma-log totals
        utri = const.tile([P, P], fp32)
        make_upper_triangular(nc, utri, val=1.0, diag=False)
        ones2 = const.tile([P, P], fp32)
        nc.gpsimd.memset(ones2[:], 1.0)

        # ---- phase 0: suffix cumprod of gammas (log space) ----
        # g16[p, i, r] = gamma[i, 16p + r]
        g16 = gam.tile([P, BPC, R16], fp32)
        nc.sync.dma_start(g16[:], gammas.rearrange("i (p r) -> p i r", r=R16))
        g16f = g16.rearrange("p i r -> p (i r)")
        nc.vector.tensor_scalar_max(g16f, g16f, GCLAMP)
        nc.scalar.activation(g16f, g16f, AF.Ln)
        incl = gam.tile([P, BPC, R16], fp32)
        zz = gam.tile([P, R16], fp32)
        nc.vector.memset(zz[:], 0.0)
        # joiner: make DVE observe the ACT (Ln) dependency before the scans
        joiner = gam.tile([P, 1], fp32)
        nc.vector.tensor_copy(out=joiner[:], in_=g16[:, 0, 0:1])
        for i in range(BPC):
            nc.vector.tensor_tensor_scan(
                incl[:, i, :], g16[:, i, :], zz[:], 0.0, OP.add, OP.add
            )
        # per-partition totals -> cross-partition exclusive prefix + full sum
        ptot = gam.tile([P, BPC], fp32)
        nc.vector.tensor_copy(out=ptot[:], in_=incl[:, :, R16 - 1])
        ps_pre = ps_w.tile([P, 2 * BPC], fp32, tag="w", name="ps_pre")
        nc.tensor.matmul(ps_pre[:, 0:BPC], utri[:], ptot[:])          # offs
        nc.tensor.matmul(ps_pre[:, BPC : 2 * BPC], ones2[:], ptot[:])  # total
        pre_sb = gam.tile([P, 2 * BPC], fp32)
        nc.vector.tensor_copy(out=pre_sb[:], in_=ps_pre[:])
        bias2 = gam.tile([P, BPC], fp32)
        nc.vector.tensor_tensor(
            bias2[:], pre_sb[:, BPC : 2 * BPC], pre_sb[:, 0:BPC], OP.subtract
        )
        # c_t[p, i, r] = exp(bias - incl) = prod_{s > 16p+r} gamma[i, s]
        c_t = gam.tile([P, BPC, R16], fp32)
        for i in range(BPC):
            nc.scalar.activation(
                c_t[:, i, :], incl[:, i, :], AF.Exp,
                bias=bias2[:, i : i + 1], scale=-1.0,
            )
        # bf16 copy of c for the 16-bit fast-path multiply
        c_bf = gam.tile([P, BPC, R16], bf16)
        nc.vector.tensor_copy(out=c_bf[:], in_=c_t[:])

        # ---- load emission: ALL inputs on the sync queue ----
        k_sb = [None] * BPC
        v_sb = [None] * BPC
        q_sb = [None] * BPC
        kb = [None] * BPC
        vcb = [None] * BPC
        qtb = [None] * BPC

        def load_k(i):
            k_sb[i] = kp.tile([P, R16, DK], fp32, tag="k", name=f"k{i}")
            nc.sync.dma_start(
                k_sb[i][:], keys[i].rearrange("(p r) k -> p r k", p=P)
            )

        def load_v(i):
            v_sb[i] = vp.tile([P, R16, DV], fp32, tag="v", name=f"v{i}")
            nc.sync.dma_start(
                v_sb[i][:], values[i].rearrange("(p r) k -> p r k", p=P)
            )

        def load_q(i):
            q_sb[i] = qp.tile([P, NQ], fp32, tag="q", name=f"q{i}")
            nc.sync.dma_start(q_sb[i][:], queriesT[i])

        # sync queue: K0 V0 K1 V1 K2 V2 Q0 K3 V3 Q1 ... K7 V7 Q5 Q6 Q7
        load_k(0); load_v(0)
        load_k(1); load_v(1)
        load_k(2); load_v(2)
        load_q(0)
        for i in range(3, BPC):
            load_k(i); load_v(i)
            load_q(i - 3)
        load_q(5); load_q(6); load_q(7)

        # ---- per-memory state tiles ----
        A_sb = [small.tile([P, P], fp32, tag=f"A{i}", name=f"A{i}") for i in range(BPC)]
        A_bf = [small.tile([P, P], bf16, tag=f"Ab{i}", name=f"Ab{i}") for i in range(BPC)]
        ST_sb = [small.tile([P, P], fp32, tag=f"S{i}", name=f"S{i}") for i in range(BPC)]
        Phi_bf = [small.tile([P, P], bf16, tag=f"Pb{i}", name=f"Phib{i}") for i in range(BPC)]
        rs_sb = [small.tile([P, 1], fp32, tag=f"r{i}", name=f"rs{i}") for i in range(BPC)]
        Xg = [None] * NGRP

        def prep(i):
            """K/V casts (Scalar/DVE by parity), bf16 c-scaling, A/S accum."""
            kb[i] = kbp.tile([P, R16, DK], bf16, tag="kb", name=f"kb{i}")
            vcb[i] = vcbp.tile([P, R16, DV], bf16, tag="vcb", name=f"vcb{i}")
            if i % 2 == 0:
                nc.scalar.copy(out=kb[i][:], in_=k_sb[i][:])
                nc.vector.tensor_copy(out=vcb[i][:], in_=v_sb[i][:])
            else:
                nc.vector.tensor_copy(out=kb[i][:], in_=k_sb[i][:])
                nc.scalar.copy(out=vcb[i][:], in_=v_sb[i][:])
            yield
            nc.vector.tensor_tensor(
                vcb[i][:], vcb[i][:],
                c_bf[:, i, :, None].to_broadcast((P, R16, DV)),
                OP.mult,
            )
            yield
            psa = ps_a.tile([P, 512], fp32, tag="a", name=f"psa{i}")
            pss = ps_s.tile([P, 512], fp32, tag="s", name=f"pss{i}")
            for r in range(R16):
                nc.tensor.matmul(
                    psa[:, 0:P], kb[i][:, r, :], kb[i][:, r, :],
                    start=(r == 0), stop=(r == R16 - 1),
                )
                nc.tensor.matmul(
                    pss[:, 0:P], kb[i][:, r, :], vcb[i][:, r, :],
                    start=(r == 0), stop=(r == R16 - 1),
                )
                if r % 2 == 1:
                    yield
            nc.vector.tensor_tensor(A_sb[i][:], psa[:, 0:P], ident[:], OP.add)
            nc.vector.tensor_copy(out=ST_sb[i][:], in_=pss[:, 0:P])
            nc.scalar.copy(out=A_bf[i][:], in_=A_sb[i][:])
            nc.vector.tensor_reduce(
                rs_sb[i][:], A_sb[i][:], AX.X, OP.add, apply_absolute_value=True
            )
            nc.vector.reciprocal(rs_sb[i][:], rs_sb[i][:])
            yield

        def x0(g):
            xw = xs.tile([P, GSZ * P], bf16, tag=f"Xb{g}", name=f"Xb{g}_0")
            for i in range(GSZ):
                nc.scalar.activation(
                    xw[:, i * P : (i + 1) * P], ident[:], AF.Copy,
                    scale=rs_sb[GSZ * g + i][:],
                )
            Xg[g] = xw

        def ns_group(g):
            """All NS iterations for pair-group g, yielding between stages."""
            for it in range(NIT):
                bf_iter = it < ns_bf
                last_bf = it == ns_bf - 1
                Amat = A_bf if bf_iter else A_sb
                pa = ps_w.tile([P, GSZ * P], fp32, tag="w", name=f"pa{g}_{it}")
                for i in range(GSZ):
                    sl = slice(i * P, (i + 1) * P)
                    nc.tensor.matmul(pa[:, sl], Amat[GSZ * g + i][:], Xg[g][:, sl])
                yield
                eg = xs.tile(
                    [P, GSZ * P], bf16 if bf_iter else fp32,
                    tag=f"e{g}_{bf_iter}", name=f"e{g}_{it}",
                )
                nc.vector.scalar_tensor_tensor(
                    eg[:], pa[:], -1.0, ident2[:], OP.mult, OP.add
                )
                yield
                pb = ps_w.tile([P, GSZ * P], fp32, tag="w", name=f"pb{g}_{it}")
                for i in range(GSZ):
                    sl = slice(i * P, (i + 1) * P)
                    nc.tensor.matmul(pb[:, sl], Xg[g][:, sl], eg[:, sl])
                yield
                out_fp32 = (not bf_iter) or last_bf
                xn = xs.tile(
                    [P, GSZ * P], fp32 if out_fp32 else bf16,
                    tag=f"Xf{g}" if out_fp32 else f"Xb{g}",
                    name=f"X{g}_{it + 1}",
                )
                nc.vector.tensor_tensor(xn[:], Xg[g][:], pb[:], OP.add)
                Xg[g] = xn
                yield

        def phi(i):
            g, sl = i // GSZ, slice((i % GSZ) * P, (i % GSZ + 1) * P)
            ps_phi = ps_w.tile([P, P], fp32, tag="w", name=f"ps_phi{i}")
            nc.tensor.matmul(ps_phi[:], Xg[g][:, sl], ST_sb[i][:])
            nc.scalar.copy(out=Phi_bf[i][:], in_=ps_phi[:])

        def ro(i):
            """Readout: cast Q^T, 4x(4 matmuls + PSUM copy), store.
            psum block s covers query rows n in [s*128, (s+1)*128):
            o_sb[m, s, v] = out[i, s*128 + m, v]."""
            qtb[i] = qbp.tile([P, NQ], bf16, tag="qtb", name=f"qtb{i}")
            h = NQ // 2
            nc.scalar.copy(out=qtb[i][:, 0:h], in_=q_sb[i][:, 0:h])
            nc.vector.tensor_copy(out=qtb[i][:, h:NQ], in_=q_sb[i][:, h:NQ])
            yield
            o_sb = outp.tile([P, R16, DV], fp32, tag="o", name=f"o{i}")
            for c in range(4):
                ps_o = ps_ro.tile([P, 4 * P], fp32, tag="ro", name=f"ps_o{i}_{c}")
                for j in range(4):
                    s = 4 * c + j
                    nc.tensor.matmul(
                        ps_o[:, j * P : (j + 1) * P],
                        qtb[i][:, s * P : (s + 1) * P], Phi_bf[i][:],
                    )
                yield
                sl = slice(4 * c, 4 * c + 4)
                if c % 2 == 0:
                    nc.vector.tensor_copy(out=o_sb[:, sl, :], in_=ps_o[:])
                else:
                    nc.scalar.copy(out=o_sb[:, sl, :], in_=ps_o[:])
                yield
            nc.scalar.dma_start(out[i][:], o_sb[:])
            yield

        def weave(*gens):
            active = [iter(g) for g in gens]
            while active:
                for g in list(active):
                    try:
                        next(g)
                    except StopIteration:
                        active.remove(g)

        chain = itertools.chain

        # ---- woven emission ----
        for _ in prep(0):
            pass
        for _ in prep(1):
            pass
        x0(0)
        weave(ns_group(0), chain(prep(2), prep(3)))
        phi(0); phi(1); x0(1)
        weave(ns_group(1), chain(prep(4), prep(5)), ro(0), ro(1))
        phi(2); phi(3); x0(2)
        weave(ns_group(2), chain(prep(6), prep(7)), ro(2), ro(3))
        phi(4); phi(5); x0(3)
        weave(ns_group(3), ro(4), ro(5))
        phi(6); phi(7)
        weave(ro(6), ro(7))

        for pool in (ps_ro, ps_w, ps_s, ps_a, xs, small, outp, qbp, qp, vcbp,
                     kbp, vp, kp, gam, const):
            pool.release()

    if not nc.is_finalized():
        nc.finalize()
    return nc


def make_in_maps(inputs):
    """Shard full inputs across cores (host-side layout transforms only)."""
    keys = np.ascontiguousarray(inputs["keys"], dtype=np.float32)
    values = np.ascontiguousarray(inputs["values"], dtype=np.float32)
    gammas = np.ascontiguousarray(inputs["gammas"], dtype=np.float32)
    queries = np.ascontiguousarray(inputs["queries"], dtype=np.float32)
    queriesT = np.ascontiguousarray(queries.transpose(0, 2, 1))
    in_maps = []
    for m in range(NCORES):
        s = slice(m * BPC, (m + 1) * BPC)
        in_maps.append(
            {
                "keys": keys[s],
                "values": values[s],
                "gammas": gammas[s],
                "queriesT": queriesT[s],
            }
        )
    return in_maps


def assemble_out(results):
    """Gather per-core outputs; un-block out_dev[i, m, s, v] -> [i, n, v]."""
    out_dev = np.concatenate([results[m]["out"] for m in range(NCORES)], axis=0)
    return np.ascontiguousarray(out_dev.transpose(0, 2, 1, 3).reshape(B, NQ, DV))


def kernel(**inputs) -> np.ndarray:
    from concourse.bass_utils import run_bass_kernel_spmd

    nc = build_nc()
    res = run_bass_kernel_spmd(
        nc, make_in_maps(inputs), core_ids=list(range(NCORES))
    )
    return assemble_out(res.results)


# revision 21
# speedup vs baseline: 1.1626x; 1.0124x over previous
"""Mesa-layer memory kernel for Trainium2 (8 NeuronCores, data-parallel over B).

Math: the reference's T-step Sherman-Morrison / discounted-accumulation
recurrence has a closed form,
    R_final = (I + K^T K)^{-1}            (eps term is O(1e-6) relative)
    S_final^T = K^T diag(c) V,   c_t = prod_{s>t} gamma_s
so per memory b the output is out_b = Q_b @ (R_b @ S_b^T).
R is computed with Newton-Schulz iterations in residual form
    X <- X + X^T (I - A X)
(bf16 iterations + one fp32 refinement; A = I + K^T K has cond ~3, so one
refinement lands at ~1e-5, far below the bf16 readout floor of ~3e-3).

v4 architecture — DMA-saturation pipeline. The kernel is HBM-bound:
33.6 MB/core at the measured ~425 GB/s per-core fabric rate = ~79 us
floor, so the design keeps the DMA queues streaming end to end and gives
every compute engine slack against the ~9.5 us/memory arrival cadence:
  * Queries are sharded host-side in TRANSPOSED layout [DK, NQ] (pure
    layout choice, same bytes moved), so the readout's Q^T operand loads
    directly: no PE transposes, no transpose-PSUM copies, and the
    readout chain is just cast -> 16 matmuls -> 4 copies -> store.
  * sync queue carries ALL input loads, interleaved K0 V0 K1 V1 K2 V2
    Q0 K3 V3 Q1 ... so each memory's K/V lead its Q by two memories;
    the scalar queue carries the 8 output stores, streaming from ~20 us.
  * V is cast to bf16 FIRST, then scaled in place by bf16(c) on DVE
    (bf16 multiply runs the fast 16-bit path; the fp32-width multiply
    measured 2.3-8 us under SBUF contention, bf16 ~0.6 us).
  * A and S^T accumulate in two separate single-bank PSUM tiles (each
    [P,512] fp32 = exactly one 2 KB zero region, so the two interleaved
    accumulation groups can never zero each other).
  * Casts alternate Scalar/DVE by memory parity so each engine's
    in-order stream follows the data-arrival order.
  * Emission is woven: each pair-group's NS iterations interleave with
    the next memories' accumulation matmuls and earlier memories'
    readout chunks.

Layout: timestep t maps to (partition p, slot r) via t = 16 p + r; every
HBM transfer is 8 KB/partition contiguous. The suffix cumprod of gammas
runs in log space: 16-step free-dim scans + one triangular matmul for the
cross-partition prefix.

Each core owns B/8 = 8 independent memories; no cross-core communication.
"""

import numpy as np

B, T, DK, DV, NQ = 64, 2048, 128, 128, 2048
NCORES = 8
BPC = B // NCORES          # memories per core
P = 128                    # partitions
R16 = T // P               # 16 row-slots per partition
GCLAMP = 1e-30             # gamma clamp before log (exact-0 gammas)

NS_BF = 5                  # Newton-Schulz iterations in bf16
NS_FP = 1                  # fp32 refinement iterations
NGRP = 4                   # NS pair-groups
GSZ = BPC // NGRP          # 2 memories per group


def build_nc(ns_bf=NS_BF, ns_fp=NS_FP):
    import itertools

    import concourse.mybir as mybir
    import concourse.tile as tile
    from concourse import bacc
    from concourse.masks import make_identity, make_upper_triangular

    fp32 = mybir.dt.float32
    bf16 = mybir.dt.bfloat16
    AF = mybir.ActivationFunctionType
    OP = mybir.AluOpType
    AX = mybir.AxisListType
    NIT = ns_bf + ns_fp

    nc = bacc.Bacc(trn_type="TRN2", target_bir_lowering=False, debug=False)
    keys = nc.dram_tensor("keys", [BPC, T, DK], fp32, kind="ExternalInput").ap()
    values = nc.dram_tensor("values", [BPC, T, DV], fp32, kind="ExternalInput").ap()
    gammas = nc.dram_tensor("gammas", [BPC, T], fp32, kind="ExternalInput").ap()
    # host-transposed query layout: [DK, NQ] per memory
    queriesT = nc.dram_tensor("queriesT", [BPC, DK, NQ], fp32, kind="ExternalInput").ap()
    # blocked output layout: out_dev[i, m, s, v] = out[i, s*128 + m, v]
    # (the host un-blocks it; pure layout transform)
    out = nc.dram_tensor("out", [BPC, P, R16, DV], fp32, kind="ExternalOutput").ap()

    with tile.TileContext(nc) as tc:
        const = tc.alloc_tile_pool(name="const", bufs=1)
        gam = tc.alloc_tile_pool(name="gam", bufs=1)
        kp = tc.alloc_tile_pool(name="kp", bufs=4)
        vp = tc.alloc_tile_pool(name="vp", bufs=4)
        kbp = tc.alloc_tile_pool(name="kbp", bufs=3)
        vcbp = tc.alloc_tile_pool(name="vcbp", bufs=3)
        qp = tc.alloc_tile_pool(name="qp", bufs=5)
        qbp = tc.alloc_tile_pool(name="qbp", bufs=3)
        outp = tc.alloc_tile_pool(name="outp", bufs=3)
        small = tc.alloc_tile_pool(name="small", bufs=1)
        xs = tc.alloc_tile_pool(name="xs", bufs=2)
        ps_a = tc.alloc_tile_pool(name="ps_a", bufs=1, space="PSUM")
        ps_s = tc.alloc_tile_pool(name="ps_s", bufs=1, space="PSUM")
        ps_w = tc.alloc_tile_pool(name="ps_w", bufs=4, space="PSUM")
        ps_ro = tc.alloc_tile_pool(name="ps_ro", bufs=2, space="PSUM")

        ident = const.tile([P, P], fp32)
        make_identity(nc, ident)
        # identity pair for the group-batched I - A@X residual
        ident2 = const.tile([P, GSZ * P], fp32)
        for i in range(GSZ):
            make_identity(nc, ident2[:, i * P : (i + 1) * P])
        # strict upper triangular and all-ones for the cross-partition
        # prefix-sum of per-partition gamma-log totals
        utri = const.tile([P, P], fp32)
        make_upper_triangular(nc, utri, val=1.0, diag=False)
        ones2 = const.tile([P, P], fp32)
        nc.gpsimd.memset(ones2[:], 1.0)

        # ---- phase 0: suffix cumprod of gammas (log space) ----
        # g16[p, i, r] = gamma[i, 16p + r]
        g16 = gam.tile([P, BPC, R16], fp32)
        nc.sync.dma_start(g16[:], gammas.rearrange("i (p r) -> p i r", r=R16))
        g16f = g16.rearrange("p i r -> p (i r)")
        nc.vector.tensor_scalar_max(g16f, g16f, GCLAMP)
        nc.scalar.activation(g16f, g16f, AF.Ln)
        incl = gam.tile([P, BPC, R16], fp32)
        zz = gam.tile([P, R16], fp32)
        nc.vector.memset(zz[:], 0.0)
        # joiner: make DVE observe the ACT (Ln) dependency before the scans
        joiner = gam.tile([P, 1], fp32)
        nc.vector.tensor_copy(out=joiner[:], in_=g16[:, 0, 0:1])
        for i in range(BPC):
            nc.vector.tensor_tensor_scan(
                incl[:, i, :], g16[:, i, :], zz[:], 0.0, OP.add, OP.add
            )
        # per-partition totals -> cross-partition exclusive prefix + full sum
        ptot = gam.tile([P, BPC], fp32)
        nc.vector.tensor_copy(out=ptot[:], in_=incl[:, :, R16 - 1])
        ps_pre = ps_w.tile([P, 2 * BPC], fp32, tag="w", name="ps_pre")
        nc.tensor.matmul(ps_pre[:, 0:BPC], utri[:], ptot[:])          # offs
        nc.tensor.matmul(ps_pre[:, BPC : 2 * BPC], ones2[:], ptot[:])  # total
        pre_sb = gam.tile([P, 2 * BPC], fp32)
        nc.vector.tensor_copy(out=pre_sb[:], in_=ps_pre[:])
        bias2 = gam.tile([P, BPC], fp32)
        nc.vector.tensor_tensor(
            bias2[:], pre_sb[:, BPC : 2 * BPC], pre_sb[:, 0:BPC], OP.subtract
        )
        # c_t[p, i, r] = exp(bias - incl) = prod_{s > 16p+r} gamma[i, s]
        c_t = gam.tile([P, BPC, R16], fp32)
        for i in range(BPC):
            nc.scalar.activation(
                c_t[:, i, :], incl[:, i, :], AF.Exp,
                bias=bias2[:, i : i + 1], scale=-1.0,
            )
        # bf16 copy of c for the 16-bit fast-path multiply
        c_bf = gam.tile([P, BPC, R16], bf16)
        nc.vector.tensor_copy(out=c_bf[:], in_=c_t[:])

        # ---- load emission: ALL inputs on the sync queue ----
        k_sb = [None] * BPC
        v_sb = [None] * BPC
        q_sb = [None] * BPC
        kb = [None] * BPC
        vcb = [None] * BPC
        qtb = [None] * BPC

        def load_k(i):
            k_sb[i] = kp.tile([P, R16, DK], fp32, tag="k", name=f"k{i}")
            nc.sync.dma_start(
                k_sb[i][:], keys[i].rearrange("(p r) k -> p r k", p=P)
            )

        def load_v(i):
            v_sb[i] = vp.tile([P, R16, DV], fp32, tag="v", name=f"v{i}")
            nc.sync.dma_start(
                v_sb[i][:], values[i].rearrange("(p r) k -> p r k", p=P)
            )

        def load_q(i):
            q_sb[i] = qp.tile([P, NQ], fp32, tag="q", name=f"q{i}")
            nc.sync.dma_start(q_sb[i][:], queriesT[i])

        # sync queue: K0 V0 K1 V1 K2 V2 Q0 K3 V3 Q1 ... K7 V7 Q5 Q6 Q7
        load_k(0); load_v(0)
        load_k(1); load_v(1)
        load_k(2); load_v(2)
        load_q(0)
        for i in range(3, BPC):
            load_k(i); load_v(i)
            load_q(i - 3)
        load_q(5); load_q(6); load_q(7)

        # ---- per-memory state tiles ----
        A_sb = [small.tile([P, P], fp32, tag=f"A{i}", name=f"A{i}") for i in range(BPC)]
        A_bf = [small.tile([P, P], bf16, tag=f"Ab{i}", name=f"Ab{i}") for i in range(BPC)]
        ST_sb = [small.tile([P, P], fp32, tag=f"S{i}", name=f"S{i}") for i in range(BPC)]
        Phi_bf = [small.tile([P, P], bf16, tag=f"Pb{i}", name=f"Phib{i}") for i in range(BPC)]
        rs_sb = [small.tile([P, 1], fp32, tag=f"r{i}", name=f"rs{i}") for i in range(BPC)]
        Xg = [None] * NGRP

        def prep(i):
            """K/V casts (Scalar/DVE by parity), bf16 c-scaling, A/S accum."""
            kb[i] = kbp.tile([P, R16, DK], bf16, tag="kb", name=f"kb{i}")
            vcb[i] = vcbp.tile([P, R16, DV], bf16, tag="vcb", name=f"vcb{i}")
            if i % 2 == 0:
                nc.scalar.copy(out=kb[i][:], in_=k_sb[i][:])
                nc.vector.tensor_copy(out=vcb[i][:], in_=v_sb[i][:])
            else:
                nc.vector.tensor_copy(out=kb[i][:], in_=k_sb[i][:])
                nc.scalar.copy(out=vcb[i][:], in_=v_sb[i][:])
            yield
            nc.vector.tensor_tensor(
                vcb[i][:], vcb[i][:],
                c_bf[:, i, :, None].to_broadcast((P, R16, DV)),
                OP.mult,
            )
            yield
            psa = ps_a.tile([P, 512], fp32, tag="a", name=f"psa{i}")
            pss = ps_s.tile([P, 512], fp32, tag="s", name=f"pss{i}")
            for r in range(R16):
                nc.tensor.matmul(
                    psa[:, 0:P], kb[i][:, r, :], kb[i][:, r, :],
                    start=(r == 0), stop=(r == R16 - 1),
                )
                nc.tensor.matmul(
                    pss[:, 0:P], kb[i][:, r, :], vcb[i][:, r, :],
                    start=(r == 0), stop=(r == R16 - 1),
                )
                if r % 2 == 1:
                    yield
            nc.vector.tensor_tensor(A_sb[i][:], psa[:, 0:P], ident[:], OP.add)
            nc.vector.tensor_copy(out=ST_sb[i][:], in_=pss[:, 0:P])
            nc.scalar.copy(out=A_bf[i][:], in_=A_sb[i][:])
            nc.vector.tensor_reduce(
                rs_sb[i][:], A_sb[i][:], AX.X, OP.add, apply_absolute_value=True
            )
            nc.vector.reciprocal(rs_sb[i][:], rs_sb[i][:])
            yield

        def x0(g):
            xw = xs.tile([P, GSZ * P], bf16, tag=f"Xb{g}", name=f"Xb{g}_0")
            for i in range(GSZ):
                nc.scalar.activation(
                    xw[:, i * P : (i + 1) * P], ident[:], AF.Copy,
                    scale=rs_sb[GSZ * g + i][:],
                )
            Xg[g] = xw

        def ns_group(g):
            """All NS iterations for pair-group g, yielding between stages."""
            for it in range(NIT):
                bf_iter = it < ns_bf
                last_bf = it == ns_bf - 1
                Amat = A_bf if bf_iter else A_sb
                pa = ps_w.tile([P, GSZ * P], fp32, tag="w", name=f"pa{g}_{it}")
                for i in range(GSZ):
                    sl = slice(i * P, (i + 1) * P)
                    nc.tensor.matmul(pa[:, sl], Amat[GSZ * g + i][:], Xg[g][:, sl])
                yield
                eg = xs.tile(
                    [P, GSZ * P], bf16 if bf_iter else fp32,
                    tag=f"e_{bf_iter}", name=f"e{g}_{it}",
                )
                nc.vector.scalar_tensor_tensor(
                    eg[:], pa[:], -1.0, ident2[:], OP.mult, OP.add
                )
                yield
                pb = ps_w.tile([P, GSZ * P], fp32, tag="w", name=f"pb{g}_{it}")
                for i in range(GSZ):
                    sl = slice(i * P, (i + 1) * P)
                    nc.tensor.matmul(pb[:, sl], Xg[g][:, sl], eg[:, sl])
                yield
                out_fp32 = (not bf_iter) or last_bf
                xn = xs.tile(
                    [P, GSZ * P], fp32 if out_fp32 else bf16,
                    tag=f"Xf{g}" if out_fp32 else f"Xb{g}",
                    name=f"X{g}_{it + 1}",
                )
                nc.vector.tensor_tensor(xn[:], Xg[g][:], pb[:], OP.add)
                Xg[g] = xn
                yield

        def phi(i):
            g, sl = i // GSZ, slice((i % GSZ) * P, (i % GSZ + 1) * P)
            ps_phi = ps_w.tile([P, P], fp32, tag="w", name=f"ps_phi{i}")
            nc.tensor.matmul(ps_phi[:], Xg[g][:, sl], ST_sb[i][:])
            nc.scalar.copy(out=Phi_bf[i][:], in_=ps_phi[:])

        def ro(i):
            """Readout: cast Q^T, 4x(4 matmuls + PSUM copy), store.
            psum block s covers query rows n in [s*128, (s+1)*128):
            o_sb[m, s, v] = out[i, s*128 + m, v]."""
            qtb[i] = qbp.tile([P, NQ], bf16, tag="qtb", name=f"qtb{i}")
            h = NQ // 2
            nc.scalar.copy(out=qtb[i][:, 0:h], in_=q_sb[i][:, 0:h])
            nc.vector.tensor_copy(out=qtb[i][:, h:NQ], in_=q_sb[i][:, h:NQ])
            yield
            o_sb = outp.tile([P, R16, DV], fp32, tag="o", name=f"o{i}")
            for c in range(4):
                ps_o = ps_ro.tile([P, 4 * P], fp32, tag="ro", name=f"ps_o{i}_{c}")
                for j in range(4):
                    s = 4 * c + j
                    nc.tensor.matmul(
                        ps_o[:, j * P : (j + 1) * P],
                        qtb[i][:, s * P : (s + 1) * P], Phi_bf[i][:],
                    )
                yield
                sl = slice(4 * c, 4 * c + 4)
                if c % 2 == 0:
                    nc.vector.tensor_copy(out=o_sb[:, sl, :], in_=ps_o[:])
                else:
                    nc.scalar.copy(out=o_sb[:, sl, :], in_=ps_o[:])
                yield
            nc.scalar.dma_start(out[i][:], o_sb[:])
            yield

        def weave(*gens):
            active = [iter(g) for g in gens]
            while active:
                for g in list(active):
                    try:
                        next(g)
                    except StopIteration:
                        active.remove(g)

        chain = itertools.chain

        # ---- woven emission ----
        for _ in prep(0):
            pass
        for _ in prep(1):
            pass
        x0(0)
        weave(ns_group(0), chain(prep(2), prep(3)))
        phi(0); phi(1); x0(1)
        weave(ro(0), ro(1), ns_group(1), chain(prep(4), prep(5)))
        phi(2); phi(3); x0(2)
        weave(ro(2), ro(3), ns_group(2), chain(prep(6), prep(7)))
        phi(4); phi(5); x0(3)
        weave(ro(4), ro(5), ns_group(3))
        phi(6); phi(7)
        weave(ro(6), ro(7))

        for pool in (ps_ro, ps_w, ps_s, ps_a, xs, small, outp, qbp, qp, vcbp,
                     kbp, vp, kp, gam, const):
            pool.release()

    if not nc.is_finalized():
        nc.finalize()
    return nc


def make_in_maps(inputs):
    """Shard full inputs across cores (host-side layout transforms only)."""
    keys = np.ascontiguousarray(inputs["keys"], dtype=np.float32)
    values = np.ascontiguousarray(inputs["values"], dtype=np.float32)
    gammas = np.ascontiguousarray(inputs["gammas"], dtype=np.float32)
    queries = np.ascontiguousarray(inputs["queries"], dtype=np.float32)
    queriesT = np.ascontiguousarray(queries.transpose(0, 2, 1))
    in_maps = []
    for m in range(NCORES):
        s = slice(m * BPC, (m + 1) * BPC)
        in_maps.append(
            {
                "keys": keys[s],
                "values": values[s],
                "gammas": gammas[s],
                "queriesT": queriesT[s],
            }
        )
    return in_maps


def assemble_out(results):
    """Gather per-core outputs; un-block out_dev[i, m, s, v] -> [i, n, v]."""
    out_dev = np.concatenate([results[m]["out"] for m in range(NCORES)], axis=0)
    return np.ascontiguousarray(out_dev.transpose(0, 2, 1, 3).reshape(B, NQ, DV))


def kernel(**inputs) -> np.ndarray:
    from concourse.bass_utils import run_bass_kernel_spmd

    nc = build_nc()
    res = run_bass_kernel_spmd(
        nc, make_in_maps(inputs), core_ids=list(range(NCORES))
    )
    return assemble_out(res.results)


# revision 24
# speedup vs baseline: 1.1750x; 1.0106x over previous
"""Mesa-layer memory kernel for Trainium2 (8 NeuronCores, data-parallel over B).

Math: the reference's T-step Sherman-Morrison / discounted-accumulation
recurrence has a closed form,
    R_final = (I + K^T K)^{-1}            (eps term is O(1e-6) relative)
    S_final^T = K^T diag(c) V,   c_t = prod_{s>t} gamma_s
so per memory b the output is out_b = Q_b @ (R_b @ S_b^T).
R is computed with Newton-Schulz iterations in residual form
    X <- X + X^T (I - A X)
(bf16 iterations + one fp32 refinement; A = I + K^T K has cond ~3, so one
refinement lands at ~1e-5, far below the bf16 readout floor of ~3e-3).

v4 architecture — DMA-saturation pipeline. The kernel is HBM-bound:
33.6 MB/core at the measured ~425 GB/s per-core fabric rate = ~79 us
floor, so the design keeps the DMA queues streaming end to end and gives
every compute engine slack against the ~9.5 us/memory arrival cadence:
  * Queries are sharded host-side in TRANSPOSED layout [DK, NQ] (pure
    layout choice, same bytes moved), so the readout's Q^T operand loads
    directly: no PE transposes, no transpose-PSUM copies, and the
    readout chain is just cast -> 16 matmuls -> 4 copies -> store.
  * sync queue carries ALL input loads, interleaved K0 V0 K1 V1 K2 V2
    Q0 K3 V3 Q1 ... so each memory's K/V lead its Q by two memories;
    the scalar queue carries the 8 output stores, streaming from ~20 us.
  * V is cast to bf16 FIRST, then scaled in place by bf16(c) on DVE
    (bf16 multiply runs the fast 16-bit path; the fp32-width multiply
    measured 2.3-8 us under SBUF contention, bf16 ~0.6 us).
  * A and S^T accumulate in two separate single-bank PSUM tiles (each
    [P,512] fp32 = exactly one 2 KB zero region, so the two interleaved
    accumulation groups can never zero each other).
  * Casts alternate Scalar/DVE by memory parity so each engine's
    in-order stream follows the data-arrival order.
  * Emission is woven: each pair-group's NS iterations interleave with
    the next memories' accumulation matmuls and earlier memories'
    readout chunks.

Layout: timestep t maps to (partition p, slot r) via t = 16 p + r; every
HBM transfer is 8 KB/partition contiguous. The suffix cumprod of gammas
runs in log space: 16-step free-dim scans + one triangular matmul for the
cross-partition prefix.

Each core owns B/8 = 8 independent memories; no cross-core communication.
"""

import numpy as np

B, T, DK, DV, NQ = 64, 2048, 128, 128, 2048
NCORES = 8
BPC = B // NCORES          # memories per core
P = 128                    # partitions
R16 = T // P               # 16 row-slots per partition
GCLAMP = 1e-30             # gamma clamp before log (exact-0 gammas)

NS_BF = 5                  # Newton-Schulz iterations in bf16
NS_FP = 1                  # fp32 refinement iterations
NGRP = 4                   # NS pair-groups
GSZ = BPC // NGRP          # 2 memories per group


def build_nc(ns_bf=NS_BF, ns_fp=NS_FP):
    import itertools

    import concourse.mybir as mybir
    import concourse.tile as tile
    from concourse import bacc
    from concourse.masks import make_identity, make_upper_triangular

    fp32 = mybir.dt.float32
    bf16 = mybir.dt.bfloat16
    AF = mybir.ActivationFunctionType
    OP = mybir.AluOpType
    AX = mybir.AxisListType
    NIT = ns_bf + ns_fp

    nc = bacc.Bacc(trn_type="TRN2", target_bir_lowering=False, debug=False)
    keys = nc.dram_tensor("keys", [BPC, T, DK], fp32, kind="ExternalInput").ap()
    values = nc.dram_tensor("values", [BPC, T, DV], fp32, kind="ExternalInput").ap()
    gammas = nc.dram_tensor("gammas", [BPC, T], fp32, kind="ExternalInput").ap()
    # host-transposed query layout: [DK, NQ] per memory
    queriesT = nc.dram_tensor("queriesT", [BPC, DK, NQ], fp32, kind="ExternalInput").ap()
    # blocked output layout: out_dev[i, m, s, v] = out[i, s*128 + m, v]
    # (the host un-blocks it; pure layout transform)
    out = nc.dram_tensor("out", [BPC, P, R16, DV], fp32, kind="ExternalOutput").ap()

    with tile.TileContext(nc) as tc:
        const = tc.alloc_tile_pool(name="const", bufs=1)
        gam = tc.alloc_tile_pool(name="gam", bufs=1)
        kp = tc.alloc_tile_pool(name="kp", bufs=4)
        vp = tc.alloc_tile_pool(name="vp", bufs=4)
        kbp = tc.alloc_tile_pool(name="kbp", bufs=3)
        vcbp = tc.alloc_tile_pool(name="vcbp", bufs=3)
        qp = tc.alloc_tile_pool(name="qp", bufs=5)
        qbp = tc.alloc_tile_pool(name="qbp", bufs=3)
        outp = tc.alloc_tile_pool(name="outp", bufs=3)
        small = tc.alloc_tile_pool(name="small", bufs=1)
        xs = tc.alloc_tile_pool(name="xs", bufs=2)
        ps_a = tc.alloc_tile_pool(name="ps_a", bufs=1, space="PSUM")
        ps_s = tc.alloc_tile_pool(name="ps_s", bufs=1, space="PSUM")
        ps_w = tc.alloc_tile_pool(name="ps_w", bufs=4, space="PSUM")
        ps_ro = tc.alloc_tile_pool(name="ps_ro", bufs=2, space="PSUM")

        ident = const.tile([P, P], fp32)
        make_identity(nc, ident)
        # identity pair for the group-batched I - A@X residual
        ident2 = const.tile([P, GSZ * P], fp32)
        for i in range(GSZ):
            make_identity(nc, ident2[:, i * P : (i + 1) * P])
        # strict upper triangular and all-ones for the cross-partition
        # prefix-sum of per-partition gamma-log totals
        utri = const.tile([P, P], fp32)
        make_upper_triangular(nc, utri, val=1.0, diag=False)
        ones2 = const.tile([P, P], fp32)
        nc.gpsimd.memset(ones2[:], 1.0)

        # ---- phase 0: suffix cumprod of gammas (log space) ----
        # g16[p, i, r] = gamma[i, 16p + r]
        g16 = gam.tile([P, BPC, R16], fp32)
        nc.sync.dma_start(g16[:], gammas.rearrange("i (p r) -> p i r", r=R16))
        g16f = g16.rearrange("p i r -> p (i r)")
        nc.vector.tensor_scalar_max(g16f, g16f, GCLAMP)
        nc.scalar.activation(g16f, g16f, AF.Ln)
        incl = gam.tile([P, BPC, R16], fp32)
        zz = gam.tile([P, R16], fp32)
        nc.vector.memset(zz[:], 0.0)
        # joiner: make DVE observe the ACT (Ln) dependency before the scans
        joiner = gam.tile([P, 1], fp32)
        nc.vector.tensor_copy(out=joiner[:], in_=g16[:, 0, 0:1])
        for i in range(BPC):
            nc.vector.tensor_tensor_scan(
                incl[:, i, :], g16[:, i, :], zz[:], 0.0, OP.add, OP.add
            )
        # per-partition totals -> cross-partition exclusive prefix + full sum
        ptot = gam.tile([P, BPC], fp32)
        nc.vector.tensor_copy(out=ptot[:], in_=incl[:, :, R16 - 1])
        ps_pre = ps_w.tile([P, 2 * BPC], fp32, tag="w", name="ps_pre")
        nc.tensor.matmul(ps_pre[:, 0:BPC], utri[:], ptot[:])          # offs
        nc.tensor.matmul(ps_pre[:, BPC : 2 * BPC], ones2[:], ptot[:])  # total
        pre_sb = gam.tile([P, 2 * BPC], fp32)
        nc.vector.tensor_copy(out=pre_sb[:], in_=ps_pre[:])
        bias2 = gam.tile([P, BPC], fp32)
        nc.vector.tensor_tensor(
            bias2[:], pre_sb[:, BPC : 2 * BPC], pre_sb[:, 0:BPC], OP.subtract
        )
        # c_t[p, i, r] = exp(bias - incl) = prod_{s > 16p+r} gamma[i, s]
        c_t = gam.tile([P, BPC, R16], fp32)
        for i in range(BPC):
            nc.scalar.activation(
                c_t[:, i, :], incl[:, i, :], AF.Exp,
                bias=bias2[:, i : i + 1], scale=-1.0,
            )
        # bf16 copy of c for the 16-bit fast-path multiply
        c_bf = gam.tile([P, BPC, R16], bf16)
        nc.vector.tensor_copy(out=c_bf[:], in_=c_t[:])

        # ---- load emission: ALL inputs on the sync queue ----
        k_sb = [None] * BPC
        v_sb = [None] * BPC
        q_sb = [None] * BPC
        kb = [None] * BPC
        vcb = [None] * BPC
        qtb = [None] * BPC

        def load_k(i):
            k_sb[i] = kp.tile([P, R16, DK], fp32, tag="k", name=f"k{i}")
            nc.sync.dma_start(
                k_sb[i][:], keys[i].rearrange("(p r) k -> p r k", p=P)
            )

        def load_v(i):
            v_sb[i] = vp.tile([P, R16, DV], fp32, tag="v", name=f"v{i}")
            nc.sync.dma_start(
                v_sb[i][:], values[i].rearrange("(p r) k -> p r k", p=P)
            )

        def load_q(i):
            q_sb[i] = qp.tile([P, NQ], fp32, tag="q", name=f"q{i}")
            nc.sync.dma_start(q_sb[i][:], queriesT[i])

        # sync queue: K0 V0 K1 V1 K2 V2 Q0 K3 V3 Q1 ... K7 V7 Q5 Q6 Q7
        load_k(0); load_v(0)
        load_k(1); load_v(1)
        load_k(2); load_v(2)
        load_q(0)
        for i in range(3, BPC):
            load_k(i); load_v(i)
            load_q(i - 3)
        load_q(5); load_q(6); load_q(7)

        # ---- per-memory state tiles ----
        A_sb = [small.tile([P, P], fp32, tag=f"A{i}", name=f"A{i}") for i in range(BPC)]
        A_bf = [small.tile([P, P], bf16, tag=f"Ab{i}", name=f"Ab{i}") for i in range(BPC)]
        ST_sb = [small.tile([P, P], fp32, tag=f"S{i}", name=f"S{i}") for i in range(BPC)]
        Phi_bf = [small.tile([P, P], bf16, tag=f"Pb{i}", name=f"Phib{i}") for i in range(BPC)]
        rs_sb = [small.tile([P, 1], fp32, tag=f"r{i}", name=f"rs{i}") for i in range(BPC)]
        Xg = [None] * NGRP

        def prep(i):
            """K/V casts (Scalar/DVE by parity), bf16 c-scaling, A/S accum."""
            kb[i] = kbp.tile([P, R16, DK], bf16, tag="kb", name=f"kb{i}")
            vcb[i] = vcbp.tile([P, R16, DV], bf16, tag="vcb", name=f"vcb{i}")
            if i % 2 == 0:
                nc.scalar.copy(out=kb[i][:], in_=k_sb[i][:])
                nc.vector.tensor_copy(out=vcb[i][:], in_=v_sb[i][:])
            else:
                nc.vector.tensor_copy(out=kb[i][:], in_=k_sb[i][:])
                nc.scalar.copy(out=vcb[i][:], in_=v_sb[i][:])
            yield
            nc.vector.tensor_tensor(
                vcb[i][:], vcb[i][:],
                c_bf[:, i, :, None].to_broadcast((P, R16, DV)),
                OP.mult,
            )
            yield
            psa = ps_a.tile([P, 512], fp32, tag="a", name=f"psa{i}")
            pss = ps_s.tile([P, 512], fp32, tag="s", name=f"pss{i}")
            for r in range(R16):
                nc.tensor.matmul(
                    psa[:, 0:P], kb[i][:, r, :], kb[i][:, r, :],
                    start=(r == 0), stop=(r == R16 - 1),
                )
                nc.tensor.matmul(
                    pss[:, 0:P], kb[i][:, r, :], vcb[i][:, r, :],
                    start=(r == 0), stop=(r == R16 - 1),
                )
                if r % 2 == 1:
                    yield
            nc.vector.tensor_tensor(A_sb[i][:], psa[:, 0:P], ident[:], OP.add)
            nc.vector.tensor_copy(out=ST_sb[i][:], in_=pss[:, 0:P])
            nc.scalar.copy(out=A_bf[i][:], in_=A_sb[i][:])
            nc.vector.tensor_reduce(
                rs_sb[i][:], A_sb[i][:], AX.X, OP.add, apply_absolute_value=True
            )
            nc.vector.reciprocal(rs_sb[i][:], rs_sb[i][:])
            yield

        def x0(g):
            xw = xs.tile([P, GSZ * P], bf16, tag=f"Xb{g}", name=f"Xb{g}_0")
            for i in range(GSZ):
                nc.scalar.activation(
                    xw[:, i * P : (i + 1) * P], ident[:], AF.Copy,
                    scale=rs_sb[GSZ * g + i][:],
                )
            Xg[g] = xw

        def ns_group(g):
            """All NS iterations for pair-group g, yielding between stages."""
            for it in range(NIT):
                bf_iter = it < ns_bf
                last_bf = it == ns_bf - 1
                Amat = A_bf if bf_iter else A_sb
                pa = ps_w.tile([P, GSZ * P], fp32, tag="w", name=f"pa{g}_{it}")
                for i in range(GSZ):
                    sl = slice(i * P, (i + 1) * P)
                    nc.tensor.matmul(pa[:, sl], Amat[GSZ * g + i][:], Xg[g][:, sl])
                yield
                eg = xs.tile(
                    [P, GSZ * P], bf16 if bf_iter else fp32,
                    tag=f"e_{bf_iter}", name=f"e{g}_{it}",
                )
                nc.vector.scalar_tensor_tensor(
                    eg[:], pa[:], -1.0, ident2[:], OP.mult, OP.add
                )
                yield
                pb = ps_w.tile([P, GSZ * P], fp32, tag="w", name=f"pb{g}_{it}")
                for i in range(GSZ):
                    sl = slice(i * P, (i + 1) * P)
                    nc.tensor.matmul(pb[:, sl], Xg[g][:, sl], eg[:, sl])
                yield
                out_fp32 = (not bf_iter) or last_bf
                xn = xs.tile(
                    [P, GSZ * P], fp32 if out_fp32 else bf16,
                    tag=f"Xf{g}" if out_fp32 else f"Xb{g}",
                    name=f"X{g}_{it + 1}",
                )
                nc.vector.tensor_tensor(xn[:], Xg[g][:], pb[:], OP.add)
                Xg[g] = xn
                yield
            for i in range(GSZ * g, GSZ * g + GSZ):
                phi(i)
            yield

        def phi(i):
            g, sl = i // GSZ, slice((i % GSZ) * P, (i % GSZ + 1) * P)
            ps_phi = ps_w.tile([P, P], fp32, tag="w", name=f"ps_phi{i}")
            nc.tensor.matmul(ps_phi[:], Xg[g][:, sl], ST_sb[i][:])
            nc.scalar.copy(out=Phi_bf[i][:], in_=ps_phi[:])

        def ro(i):
            """Readout: cast Q^T, 4x(4 matmuls + PSUM copy), store.
            psum block s covers query rows n in [s*128, (s+1)*128):
            o_sb[m, s, v] = out[i, s*128 + m, v]."""
            qtb[i] = qbp.tile([P, NQ], bf16, tag="qtb", name=f"qtb{i}")
            h = NQ // 2
            nc.scalar.copy(out=qtb[i][:, 0:h], in_=q_sb[i][:, 0:h])
            nc.vector.tensor_copy(out=qtb[i][:, h:NQ], in_=q_sb[i][:, h:NQ])
            yield
            o_sb = outp.tile([P, R16, DV], fp32, tag="o", name=f"o{i}")
            for c in range(4):
                ps_o = ps_ro.tile([P, 4 * P], fp32, tag="ro", name=f"ps_o{i}_{c}")
                for j in range(4):
                    s = 4 * c + j
                    nc.tensor.matmul(
                        ps_o[:, j * P : (j + 1) * P],
                        qtb[i][:, s * P : (s + 1) * P], Phi_bf[i][:],
                    )
                yield
                sl = slice(4 * c, 4 * c + 4)
                if c % 2 == 0:
                    nc.vector.tensor_copy(out=o_sb[:, sl, :], in_=ps_o[:])
                else:
                    nc.scalar.copy(out=o_sb[:, sl, :], in_=ps_o[:])
                yield
            nc.scalar.dma_start(out[i][:], o_sb[:])
            yield

        def weave(*gens):
            """Round-robin generators; (gen, w) advances w yields per round."""
            active = []
            for x in gens:
                g, w = x if isinstance(x, tuple) else (x, 1)
                active.append([iter(g), w])
            while active:
                for a in list(active):
                    try:
                        for _ in range(a[1]):
                            next(a[0])
                    except StopIteration:
                        active.remove(a)

        chain = itertools.chain

        # ---- woven emission ----
        for _ in prep(0):
            pass
        for _ in prep(1):
            pass
        x0(0)
        weave((ns_group(0), 3), chain(prep(2), prep(3)))
        x0(1)
        weave((ns_group(1), 3), (ro(0), 2), (ro(1), 2),
              chain(prep(4), prep(5)))
        x0(2)
        weave((ns_group(2), 3), (ro(2), 2), (ro(3), 2),
              chain(prep(6), prep(7)))
        x0(3)
        weave((ns_group(3), 3), (ro(4), 2), (ro(5), 2))
        weave(ro(6), ro(7))

        for pool in (ps_ro, ps_w, ps_s, ps_a, xs, small, outp, qbp, qp, vcbp,
                     kbp, vp, kp, gam, const):
            pool.release()

    if not nc.is_finalized():
        nc.finalize()
    return nc


def make_in_maps(inputs):
    """Shard full inputs across cores (host-side layout transforms only)."""
    keys = np.ascontiguousarray(inputs["keys"], dtype=np.float32)
    values = np.ascontiguousarray(inputs["values"], dtype=np.float32)
    gammas = np.ascontiguousarray(inputs["gammas"], dtype=np.float32)
    queries = np.ascontiguousarray(inputs["queries"], dtype=np.float32)
    queriesT = np.ascontiguousarray(queries.transpose(0, 2, 1))
    in_maps = []
    for m in range(NCORES):
        s = slice(m * BPC, (m + 1) * BPC)
        in_maps.append(
            {
                "keys": keys[s],
                "values": values[s],
                "gammas": gammas[s],
                "queriesT": queriesT[s],
            }
        )
    return in_maps


def assemble_out(results):
    """Gather per-core outputs; un-block out_dev[i, m, s, v] -> [i, n, v]."""
    out_dev = np.concatenate([results[m]["out"] for m in range(NCORES)], axis=0)
    return np.ascontiguousarray(out_dev.transpose(0, 2, 1, 3).reshape(B, NQ, DV))


def kernel(**inputs) -> np.ndarray:
    from concourse.bass_utils import run_bass_kernel_spmd

    nc = build_nc()
    res = run_bass_kernel_spmd(
        nc, make_in_maps(inputs), core_ids=list(range(NCORES))
    )
    return assemble_out(res.results)
